# revision 32
# baseline (speedup 1.0000x reference)
"""Trainium2 Bass kernel for classical causal MHA (B=2, T=2048, D=1024, H=16).

Sharding: 8 cores = 2 batches x 4 head-groups (4 heads / 256 dims each).
Each core computes QKV projections for its head-group, causal attention,
and a partial output projection; the host sums the 4 partials per batch
and adds the (bv @ Wo + bo) correction (the v-bias commutes through
softmax-weighted averaging, so it is applied after the kernel).

Design (HW-measured ~97us/rep steady state vs 219us baseline):
- All matmul operands are bf16 (1 cycle/row at any free size, FWL halves
  weight loads, DMA bytes halved). PSUM accumulation stays fp32. NOTE:
  fp16 operands measure ~3x SLOWER on real HW despite the cost model
  rating them equal — do not switch. The partial-output DMA is fp16
  (safe: 10 mantissa bits, values O(1)).
- Score matmuls pack the 2 heads of a pair on disjoint 64-row PE groups
  (auto tile_position) — they genuinely overlap on HW.
- Chunk-pipelined schedule: projections for q-chunk c+1 and the output
  projection for chunk c-1 are split into ~0.4-0.9us quanta and threaded
  between attention blocks of chunk c, so the PE never sits behind the
  activation engine's exp chain. The last chunk's S/exp blocks are
  prefetched into persistent tiles during chunk 2; its PV runs as a
  pure-PE chase at the end (p=1 chain first so the partition-shift DMA
  of its normalize hides under p=0's chain).
- Causal trim: S and PV matmuls stream only the [zc:512) live columns of
  diagonal blocks; the in-window upper triangle is zeroed by a single
  gpsimd affine_select per diagonal block (no masks, no memsets, no DVE
  mask-muls).
- Input DMAs are spread over the sync/scalar/gpsimd queues in dependency
  order; warm-up matmuls (first rep only) keep the PE HAM at 8/8 during
  the initial loads.
- The softmax denominator comes from a ones column appended to V in the
  PV matmul; no max-subtraction is needed because scores are O(1).
"""

import os
import sys

for _p in ("/opt/trn_rl_repo", "/root/.axon_site/_ro/trn_rl_repo"):
    if os.path.isdir(_p) and _p not in sys.path:
        sys.path.insert(0, _p)

import numpy as np

D = 1024
NH = 16
HD = 64
NCORES = 8
GROUPS = 4          # head-groups per batch
HLOC = NH // GROUPS  # heads per core
CW = HLOC * HD       # per-core projection width (256)
SCALE = 1.0 / float(np.sqrt(HD))

_CACHE = {}


def build_nc(T, repeat=1, cfg=None):
    cfg = dict(cfg or {})
    PSA = cfg.get("psA", 2)
    PSS = cfg.get("psS", 2)
    PSPV = cfg.get("psPV", 1)
    RING = cfg.get("ring", 4)
    LA = cfg.get("la", 3)
    WARM = cfg.get("warm", 10)
    import concourse.tile as tile
    from concourse import bacc, mybir

    f32 = mybir.dt.float32
    bf = mybir.dt.bfloat16
    f16 = mybir.dt.float16
    AF = mybir.ActivationFunctionType

    QCH = min(512, T)     # q-chunk width
    NQ = T // QCH
    RB = QCH // 128       # k-blocks per q-chunk
    TB = T // 128
    NK = D // 128         # contraction chunks for projections

    nc = bacc.Bacc(None, target_bir_lowering=False, debug=False)
    xT_d = nc.dram_tensor("xT", [D, T], bf, kind="ExternalInput")
    wq_d = nc.dram_tensor("wq", [D, CW], bf, kind="ExternalInput")
    wk_d = nc.dram_tensor("wk", [D, CW], bf, kind="ExternalInput")
    wv_d = nc.dram_tensor("wv", [D, CW], bf, kind="ExternalInput")
    wo_d = nc.dram_tensor("wo", [CW, D], bf, kind="ExternalInput")
    bq_d = nc.dram_tensor("bq2", [128, CW // 128], f32, kind="ExternalInput")
    bk_d = nc.dram_tensor("bk2", [128, CW // 128], f32, kind="ExternalInput")
    out_d = nc.dram_tensor("out", [T, D], f16, kind="ExternalOutput")

    with tile.TileContext(nc) as tc:
        from contextlib import ExitStack

        for _rep in range(repeat):
          with ExitStack() as es:
            pers = es.enter_context(tc.tile_pool(name=f"pers{_rep}", bufs=1))
            psA = es.enter_context(tc.tile_pool(name=f"psA{_rep}", bufs=PSA, space="PSUM"))
            psS = es.enter_context(tc.tile_pool(name=f"psS{_rep}", bufs=PSS, space="PSUM"))
            psPV = es.enter_context(tc.tile_pool(name=f"psPV{_rep}", bufs=PSPV, space="PSUM"))
            ring = es.enter_context(tc.tile_pool(name=f"ring{_rep}", bufs=RING))
            small = es.enter_context(tc.tile_pool(name=f"small{_rep}", bufs=2))

            qT_sb = pers.tile([128, 2, T], bf, tag="qT")
            kT_sb = pers.tile([128, 2, T], bf, tag="kT")
            attn_q = [
                pers.tile([128, 2, QCH], bf, tag=f"attn{jq}", name=f"attnq{jq}")
                for jq in range(NQ)
            ]
            v1_sb = pers.tile([128, TB, HLOC, HD + 1], bf, tag="v1")
            bq_sb = pers.tile([128, CW // 128], f32, tag="bq")
            bk_sb = pers.tile([128, CW // 128], f32, tag="bk")
            wo_sb = pers.tile([128, 2, D], bf, tag="wo")
            warm_sb = pers.tile([128, 256], bf, tag="warm")
            xc = [
                pers.tile([128, NK, QCH], bf, tag=f"xc{c}", name=f"xc{c}")
                for c in range(NQ)
            ]
            wq_sb = pers.tile([128, NK, CW], bf, tag="wq")
            wk_sb = pers.tile([128, NK, CW], bf, tag="wk")
            wv_sb = pers.tile([128, NK, CW], bf, tag="wv")

            # --- input DMAs, spread across queues in dependency order ---
            def _ld_x(c, eng):
                eng.dma_start(
                    xc[c][:],
                    xT_d[:, QCH * c : QCH * (c + 1)].rearrange(
                        "(k p) t -> p k t", p=128
                    ),
                )

            def _ld_w(w_sb, w_d, eng):
                eng.dma_start(
                    w_sb[:], w_d[:].rearrange("(k p) w -> p k w", p=128)
                )

            # first Q-projection group only needs wq's m=0 half and the low
            # kk-half of x chunk 0 — split those DMAs so compute starts ~2us
            # earlier on a cold dispatch
            nc.sync.dma_start(bq_sb[:], bq_d[:])
            nc.scalar.dma_start(
                wq_sb[:, :, 0:128],
                wq_d[:, 0:128].rearrange("(k p) w -> p k w", p=128),
            )
            nc.gpsimd.dma_start(bk_sb[:], bk_d[:])
            nc.sync.dma_start(
                xc[0][:, 0:4, :],
                xT_d[0:512, 0:QCH].rearrange("(k p) t -> p k t", p=128),
            )
            nc.scalar.dma_start(
                wq_sb[:, :, 128:256],
                wq_d[:, 128:256].rearrange("(k p) w -> p k w", p=128),
            )
            nc.sync.dma_start(
                xc[0][:, 4:8, :],
                xT_d[512:1024, 0:QCH].rearrange("(k p) t -> p k t", p=128),
            )
            _ld_w(wk_sb, wk_d, nc.scalar)
            _ld_w(wv_sb, wv_d, nc.gpsimd)
            _ld_x(1, nc.sync)
            _ld_x(2, nc.sync)
            _ld_x(3, nc.gpsimd)
            for m in range(2):
                nc.gpsimd.dma_start(wo_sb[:, m, :], wo_d[128 * m : 128 * (m + 1), :])

            # ones column of v1 (bf16)
            nc.vector.memset(v1_sb[:, :, :, HD : HD + 1], 1.0)

            # --- PE warm-up during the initial loads (keeps HAM at 8/8).
            # Only the first dispatch starts cold; later reps inherit a warm
            # PE, so don't pay for it in steady state.
            if _rep == 0:
                nc.vector.memset(warm_sb[:], 0.0)
                for w in range(WARM):
                    pw = psA.tile([128, 256], f32, tag="pa", name=f"warm{w}")
                    nc.tensor.matmul(
                        pw[:], warm_sb[:, 0:128], warm_sb[:], start=True,
                        stop=True,
                    )

            # --- work quanta -------------------------------------------------
            def qk_quanta(m, which, jc):
                dst, w_sb, b_sb = (
                    (qT_sb, wq_sb, bq_sb) if which == 0 else (kT_sb, wk_sb, bk_sb)
                )
                hold = {}

                def make(kp):
                    def q():
                        if kp == 0:
                            hold["pp"] = psA.tile(
                                [128, QCH], f32, tag="pa",
                                name=f"pp{m}{which}{jc}",
                            )
                        pp = hold["pp"]
                        for kk in (2 * kp, 2 * kp + 1):
                            nc.tensor.matmul(
                                pp[:],
                                w_sb[:, kk, 128 * m : 128 * (m + 1)],
                                xc[jc][:, kk, :],
                                start=(kk == 0),
                                stop=(kk == NK - 1),
                            )
                        if kp == 3:
                            nc.vector.tensor_scalar_add(
                                dst[:, m, QCH * jc : QCH * (jc + 1)],
                                pp[:],
                                b_sb[:, m : m + 1],
                            )
                    return q

                return [make(kp) for kp in range(4)]

            def v_quanta(tb):
                hold = {}

                def make(h):
                    def q():
                        if h == 0:
                            hold["pv"] = psA.tile(
                                [128, CW], f32, tag="pa", name=f"pv{tb}"
                            )
                        pv = hold["pv"]
                        for kk in range(4 * h, 4 * h + 4):
                            nc.tensor.matmul(
                                pv[:],
                                xc[tb // RB][:, kk, 128 * (tb % RB) : 128 * (tb % RB + 1)],
                                wv_sb[:, kk, :],
                                start=(kk == 0),
                                stop=(kk == NK - 1),
                            )
                        if h == 1:
                            nc.vector.tensor_copy(
                                v1_sb[:, tb, :, 0:HD],
                                pv[:].rearrange("p (h d) -> p h d", h=HLOC),
                            )
                    return q

                return [make(h) for h in range(2)]

            def o_quanta(tb, act_evac=False):
                t_sl = slice(128 * tb, 128 * (tb + 1))
                hold = {}

                def make(n):
                    def q():
                        if n == 0:
                            hold["o"] = small.tile(
                                [128, D], f16, tag="osb", bufs=2, name=f"osb{tb}"
                            )
                        o_t = hold["o"]
                        po = psA.tile(
                            [128, 512], f32, tag="pa", name=f"po{tb}{n}"
                        )
                        for m in range(2):
                            nc.tensor.matmul(
                                po[:],
                                attn_q[tb // RB][:, m, 128 * (tb % RB) : 128 * (tb % RB + 1)],
                                wo_sb[:, m, 512 * n : 512 * (n + 1)],
                                start=(m == 0),
                                stop=(m == 1),
                            )
                        if act_evac and n == 1:
                            nc.scalar.copy(o_t[:, 512 * n : 512 * (n + 1)], po[:])
                        else:
                            nc.vector.tensor_copy(
                                o_t[:, 512 * n : 512 * (n + 1)], po[:]
                            )
                        eng = nc.sync if tb % 2 == 0 else nc.gpsimd
                        if act_evac:
                            # tail chunks: ship each half as soon as it is
                            # evacuated so the last DMA drains sooner
                            eng.dma_start(
                                out_d[t_sl, 512 * n : 512 * (n + 1)],
                                o_t[:, 512 * n : 512 * (n + 1)],
                            )
                        elif n == 1:
                            eng.dma_start(out_d[t_sl, :], o_t[:])
                    return q

                return [make(n) for n in range(2)]

            def unit_quanta(c):
                qs = []
                qs += qk_quanta(0, 0, c)
                qs += qk_quanta(0, 1, c)
                for tb in range(RB * c, RB * (c + 1)):
                    qs += v_quanta(tb)
                qs += qk_quanta(1, 0, c)
                qs += qk_quanta(1, 1, c)
                return qs

            # --- attention for (m, j), threading filler quanta between blocks
            def zc_of(i, j):
                ri = i - RB * j
                return 128 * ri if ri > 0 else 0

            def emit_s_block(m, j, i, pt):
                ri = i - RB * j
                zc = zc_of(i, j)
                sp = psS.tile([128, 2, QCH], f32, tag="s",
                              name=f"s{m}{j}{i}")
                for p in range(2):
                    hsl = slice(64 * p, 64 * (p + 1))
                    nc.tensor.matmul(
                        sp[:, p, zc:QCH],
                        kT_sb[hsl, m, 128 * i : 128 * (i + 1)],
                        qT_sb[hsl, m, QCH * j + zc : QCH * (j + 1)],
                        start=True,
                        stop=True,
                    )
                nc.scalar.activation(
                    pt[:, :, zc:QCH], sp[:, :, zc:QCH], AF.Exp, scale=SCALE
                )
                if ri >= 0:
                    # zero the in-window upper triangle: keep where q >= k
                    nc.gpsimd.affine_select(
                        out=pt[:, :, zc : zc + 128],
                        in_=pt[:, :, zc : zc + 128],
                        compare_op=mybir.AluOpType.is_ge,
                        fill=0.0,
                        base=0,
                        pattern=[[0, 2], [1, 128]],
                        channel_multiplier=-1,
                    )

            def emit_attn(m, j, filler, pre=None):
                kb = (j + 1) * RB
                pvp = [
                    psPV.tile([128, QCH], f32, tag=f"pvac{p}",
                              name=f"pvac{m}{j}{p}")
                    for p in range(2)
                ]

                def pv_ap(p, rsl, csl):
                    return pvp[p][rsl, csl]
                pts = {}

                def emit_pv(i, ps=(0, 1)):
                    zc = zc_of(i, j)
                    for p in ps:
                        nc.tensor.matmul(
                            pv_ap(p, slice(0, HD + 1), slice(zc, QCH)),
                            v1_sb[:, i, 2 * m + p, :],
                            pts[i][:, p, zc:QCH],
                            start=(i == 0),
                            stop=(i == kb - 1),
                            skip_group_check=True,
                        )

                def normalize(p):
                    recip = small.tile([1, QCH], f32, tag="recip",
                                       name=f"rc{m}{j}{p}")
                    nc.vector.reciprocal(recip[:], pv_ap(p, slice(HD, HD + 1),
                                                         slice(0, QCH)))
                    bcast = small.tile([64, QCH], f32, tag="bcast", bufs=2,
                                       name=f"bc{m}{j}{p}")
                    nc.gpsimd.partition_broadcast(bcast[:], recip[:])
                    pv_body = pv_ap(p, slice(0, HD), slice(0, QCH))
                    if p == 0:
                        nc.vector.tensor_mul(
                            attn_q[j][0:64, m, :], pv_body, bcast[:]
                        )
                    else:
                        tmp = small.tile([64, QCH], bf, tag="tmp",
                                         name=f"tmp{m}{j}")
                        nc.vector.tensor_mul(tmp[:], pv_body, bcast[:])
                        nc.gpsimd.dma_start(attn_q[j][64:128, m, :], tmp[:])

                if pre is not None:
                    # S/exp already done into persistent tiles: pure PV chase.
                    # p=1 chain first so its normalize + partition-shift DMA
                    # hides under p=0's chain.
                    for i in range(kb):
                        pts[i] = pre[i]
                    for p in (1, 0):
                        for i in range(kb):
                            if i % 4 == 0:
                                f = next(filler, None)
                                if f is not None:
                                    f()
                            emit_pv(i, ps=(p,))
                        normalize(p)
                    return
                else:
                    for i in range(kb):
                        pt = ring.tile([128, 2, QCH], bf, tag="pt",
                                       name=f"pt{m}{j}{i}")
                        emit_s_block(m, j, i, pt)
                        pts[i] = pt
                        f = next(filler, None)
                        if f is not None:
                            f()
                        if i >= LA:
                            emit_pv(i - LA)
                    for i in range(max(0, kb - LA), kb):
                        emit_pv(i)

                for p in (1, 0):
                    normalize(p)

            # --- schedule ----------------------------------------------------
            from itertools import chain

            JL = NQ - 1  # last chunk: S/exp prefetched, PV chased at the end
            pt3 = [
                [
                    pers.tile([128, 2, QCH], bf, tag=f"pt3_{m}_{i}",
                              name=f"pt3_{m}_{i}")
                    for i in range(RB * NQ)
                ]
                for m in range(2)
            ]

            def s3_quanta(m):
                def make(i):
                    return lambda: emit_s_block(m, JL, i, pt3[m][i])
                return [make(i) for i in range(RB * (JL + 1))]

            def o_all(c, act_evac=False):
                return [q for tb in range(RB * c, RB * (c + 1))
                        for q in o_quanta(tb, act_evac)]

            def weave(a, b, ratio=2):
                out, ia, ib = [], 0, 0
                while ia < len(a) or ib < len(b):
                    for _ in range(ratio):
                        if ia < len(a):
                            out.append(a[ia])
                            ia += 1
                    if ib < len(b):
                        out.append(b[ib])
                        ib += 1
                return out

            for q in unit_quanta(0):
                q()
            for c in range(NQ - 1):
                fill = [unit_quanta(c + 1)]
                if c + 1 == JL:
                    # weave in the last chunk's S/exp blocks right after its
                    # Q/K projections so the Act engine never goes idle
                    uq = fill[0]
                    fill = [uq[:8], s3_quanta(0), uq[8:],
                            weave(s3_quanta(1), o_all(c - 1) if c >= 1 else [])]
                elif c >= 1:
                    fill.append(o_all(c - 1))
                filler = chain(*fill)
                emit_attn(0, c, filler)
                emit_attn(1, c, filler)
                for f in filler:
                    f()
            filler = chain(o_all(JL - 1, act_evac=True))
            emit_attn(0, JL, filler, pre=pt3[0])
            emit_attn(1, JL, filler, pre=pt3[1])
            for f in filler:
                f()
            for q in o_all(JL, act_evac=True):
                q()

    nc.compile()
    return nc


def shard_inputs(x, Wq, bq, Wk, bk, Wv, Wo, bf16_in=True):
    import ml_dtypes

    bfi = ml_dtypes.bfloat16
    in_maps = []
    for c in range(NCORES):
        b, g = divmod(c, GROUPS)
        cols = slice(g * CW, (g + 1) * CW)
        in_maps.append(
            {
                "xT": np.ascontiguousarray(x[b].T).astype(bfi),
                "wq": np.ascontiguousarray(Wq[:, cols]).astype(bfi),
                "wk": np.ascontiguousarray(Wk[:, cols]).astype(bfi),
                "wv": np.ascontiguousarray(Wv[:, cols]).astype(bfi),
                "wo": np.ascontiguousarray(Wo[cols, :]).astype(bfi),
                "bq2": np.ascontiguousarray(bq[cols].reshape(CW // 128, 128).T),
                "bk2": np.ascontiguousarray(bk[cols].reshape(CW // 128, 128).T),
            }
        )
    return in_maps


def gather_outputs(results, x, Wv_b, Wo, bo, bv):
    B, T, _ = x.shape
    y = np.empty((B, T, D), np.float32)
    corr = (bv @ Wo + bo).astype(np.float32)
    for b in range(B):
        acc = results[GROUPS * b]["out"].astype(np.float32)
        for g in range(1, GROUPS):
            acc += results[GROUPS * b + g]["out"].astype(np.float32)
        y[b] = acc + corr
    return y


def kernel(x, Wq, bq, Wk, bk, Wv, bv, Wo, bo):
    from concourse import bass_utils

    x = np.asarray(x, np.float32)
    T = x.shape[1]
    if T not in _CACHE:
        _CACHE[T] = build_nc(T)
    nc = _CACHE[T]
    in_maps = shard_inputs(
        x,
        np.asarray(Wq, np.float32), np.asarray(bq, np.float32),
        np.asarray(Wk, np.float32), np.asarray(bk, np.float32),
        np.asarray(Wv, np.float32), np.asarray(Wo, np.float32),
    )
    res = bass_utils.run_bass_kernel_spmd(
        nc, in_maps, core_ids=list(range(NCORES))
    )
    y = gather_outputs(res.results, x, None, np.asarray(Wo, np.float32),
                       np.asarray(bo, np.float32), np.asarray(bv, np.float32))
    return y


# revision 33
# speedup vs baseline: 2.6700x; 2.6700x over previous
"""Trainium2 Bass kernel for classical causal MHA (B=2, T=2048, D=1024, H=16).

Sharding: 8 cores = 2 batches x 4 head-groups (4 heads / 256 dims each).
Each core computes QKV projections for its head-group, causal attention,
and a partial output projection; the host sums the 4 partials per batch
and adds the (bv @ Wo + bo) correction (the v-bias commutes through
softmax-weighted averaging, so it is applied after the kernel).

Design (HW-measured ~97us/rep steady state vs 219us baseline):
- All matmul operands are bf16 (1 cycle/row at any free size, FWL halves
  weight loads, DMA bytes halved). PSUM accumulation stays fp32. NOTE:
  fp16 operands measure ~3x SLOWER on real HW despite the cost model
  rating them equal — do not switch. The partial-output DMA is fp16
  (safe: 10 mantissa bits, values O(1)).
- Score matmuls pack the 2 heads of a pair on disjoint 64-row PE groups
  (auto tile_position) — they genuinely overlap on HW.
- Chunk-pipelined schedule: projections for q-chunk c+1 and the output
  projection for chunk c-1 are split into ~0.4-0.9us quanta and threaded
  between attention blocks of chunk c, so the PE never sits behind the
  activation engine's exp chain. The last chunk's S/exp blocks are
  prefetched into persistent tiles during chunk 2; its PV runs as a
  pure-PE chase at the end (p=1 chain first so the partition-shift DMA
  of its normalize hides under p=0's chain).
- Causal trim: S and PV matmuls stream only the [zc:512) live columns of
  diagonal blocks; the in-window upper triangle is zeroed by a single
  gpsimd affine_select per diagonal block (no masks, no memsets, no DVE
  mask-muls).
- Input DMAs are spread over the sync/scalar/gpsimd queues in dependency
  order; warm-up matmuls (first rep only) keep the PE HAM at 8/8 during
  the initial loads.
- The softmax denominator comes from a ones column appended to V in the
  PV matmul; no max-subtraction is needed because scores are O(1).
"""

import os
import sys

for _p in ("/opt/trn_rl_repo", "/root/.axon_site/_ro/trn_rl_repo"):
    if os.path.isdir(_p) and _p not in sys.path:
        sys.path.insert(0, _p)

import numpy as np

D = 1024
NH = 16
HD = 64
NCORES = 8
GROUPS = 4          # head-groups per batch
HLOC = NH // GROUPS  # heads per core
CW = HLOC * HD       # per-core projection width (256)
SCALE = 1.0 / float(np.sqrt(HD))

_CACHE = {}


def build_nc(T, repeat=1, cfg=None):
    cfg = dict(cfg or {})
    PSA = cfg.get("psA", 2)
    PSS = cfg.get("psS", 2)
    PSPV = cfg.get("psPV", 1)
    RING = cfg.get("ring", 4)
    LA = cfg.get("la", 3)
    WARM = cfg.get("warm", 10)
    import concourse.tile as tile
    from concourse import bacc, mybir

    f32 = mybir.dt.float32
    bf = mybir.dt.bfloat16
    f16 = mybir.dt.float16
    AF = mybir.ActivationFunctionType

    QCH = min(512, T)     # q-chunk width
    NQ = T // QCH
    RB = QCH // 128       # k-blocks per q-chunk
    TB = T // 128
    NK = D // 128         # contraction chunks for projections

    nc = bacc.Bacc(None, target_bir_lowering=False, debug=False)
    xT_d = nc.dram_tensor("xT", [D, T], bf, kind="ExternalInput")
    wq_d = nc.dram_tensor("wq", [D, CW], bf, kind="ExternalInput")
    wk_d = nc.dram_tensor("wk", [D, CW], bf, kind="ExternalInput")
    wv_d = nc.dram_tensor("wv", [D, CW], bf, kind="ExternalInput")
    wo_d = nc.dram_tensor("wo", [CW, D], bf, kind="ExternalInput")
    bq_d = nc.dram_tensor("bq2", [128, CW // 128], f32, kind="ExternalInput")
    bk_d = nc.dram_tensor("bk2", [128, CW // 128], f32, kind="ExternalInput")
    out_d = nc.dram_tensor("out", [T, D], f16, kind="ExternalOutput")

    with tile.TileContext(nc) as tc:
        from contextlib import ExitStack

        for _rep in range(repeat):
          with ExitStack() as es:
            pers = es.enter_context(tc.tile_pool(name=f"pers{_rep}", bufs=1))
            psA = es.enter_context(tc.tile_pool(name=f"psA{_rep}", bufs=PSA, space="PSUM"))
            psS = es.enter_context(tc.tile_pool(name=f"psS{_rep}", bufs=PSS, space="PSUM"))
            psPV = es.enter_context(tc.tile_pool(name=f"psPV{_rep}", bufs=PSPV, space="PSUM"))
            ring = es.enter_context(tc.tile_pool(name=f"ring{_rep}", bufs=RING))
            small = es.enter_context(tc.tile_pool(name=f"small{_rep}", bufs=2))

            qT_sb = pers.tile([128, 2, T], bf, tag="qT")
            kT_sb = pers.tile([128, 2, T], bf, tag="kT")
            attn_q = [
                pers.tile([128, 2, QCH], bf, tag=f"attn{jq}", name=f"attnq{jq}")
                for jq in range(NQ)
            ]
            v1_sb = pers.tile([128, TB, HLOC, HD + 1], bf, tag="v1")
            bq_sb = pers.tile([128, CW // 128], f32, tag="bq")
            bk_sb = pers.tile([128, CW // 128], f32, tag="bk")
            wo_sb = pers.tile([128, 2, D], bf, tag="wo")
            warm_sb = pers.tile([128, 256], bf, tag="warm")
            xc = [
                pers.tile([128, NK, QCH], bf, tag=f"xc{c}", name=f"xc{c}")
                for c in range(NQ)
            ]
            wq_sb = pers.tile([128, NK, CW], bf, tag="wq")
            wk_sb = pers.tile([128, NK, CW], bf, tag="wk")
            wv_sb = pers.tile([128, NK, CW], bf, tag="wv")

            # --- input DMAs, spread across queues in dependency order ---
            def _ld_x(c, eng):
                eng.dma_start(
                    xc[c][:],
                    xT_d[:, QCH * c : QCH * (c + 1)].rearrange(
                        "(k p) t -> p k t", p=128
                    ),
                )

            def _ld_w(w_sb, w_d, eng):
                eng.dma_start(
                    w_sb[:], w_d[:].rearrange("(k p) w -> p k w", p=128)
                )

            # first Q-projection group only needs wq's m=0 half and the low
            # kk-half of x chunk 0 — split those DMAs so compute starts ~2us
            # earlier on a cold dispatch
            nc.sync.dma_start(bq_sb[:], bq_d[:])
            nc.scalar.dma_start(
                wq_sb[:, :, 0:128],
                wq_d[:, 0:128].rearrange("(k p) w -> p k w", p=128),
            )
            nc.gpsimd.dma_start(bk_sb[:], bk_d[:])
            nc.sync.dma_start(
                xc[0][:, 0:4, :],
                xT_d[0:512, 0:QCH].rearrange("(k p) t -> p k t", p=128),
            )
            nc.scalar.dma_start(
                wq_sb[:, :, 128:256],
                wq_d[:, 128:256].rearrange("(k p) w -> p k w", p=128),
            )
            nc.sync.dma_start(
                xc[0][:, 4:8, :],
                xT_d[512:1024, 0:QCH].rearrange("(k p) t -> p k t", p=128),
            )
            _ld_w(wk_sb, wk_d, nc.scalar)
            _ld_w(wv_sb, wv_d, nc.gpsimd)
            _ld_x(1, nc.sync)
            _ld_x(2, nc.sync)
            _ld_x(3, nc.gpsimd)
            for m in range(2):
                nc.gpsimd.dma_start(wo_sb[:, m, :], wo_d[128 * m : 128 * (m + 1), :])

            # ones column of v1 (bf16)
            nc.vector.memset(v1_sb[:, :, :, HD : HD + 1], 1.0)

            # --- PE warm-up during the initial loads (keeps HAM at 8/8).
            # Only the first dispatch starts cold; later reps inherit a warm
            # PE, so don't pay for it in steady state.
            if _rep == 0:
                nc.vector.memset(warm_sb[:], 0.0)
                for w in range(WARM):
                    pw = psA.tile([128, 256], f32, tag="pa", name=f"warm{w}")
                    nc.tensor.matmul(
                        pw[:], warm_sb[:, 0:128], warm_sb[:], start=True,
                        stop=True,
                    )

            # --- work quanta -------------------------------------------------
            def qk_quanta(m, which, jc):
                dst, w_sb, b_sb = (
                    (qT_sb, wq_sb, bq_sb) if which == 0 else (kT_sb, wk_sb, bk_sb)
                )
                hold = {}

                def make(kp):
                    def q():
                        if kp == 0:
                            hold["pp"] = psA.tile(
                                [128, QCH], f32, tag="pa",
                                name=f"pp{m}{which}{jc}",
                            )
                        pp = hold["pp"]
                        for kk in (2 * kp, 2 * kp + 1):
                            nc.tensor.matmul(
                                pp[:],
                                w_sb[:, kk, 128 * m : 128 * (m + 1)],
                                xc[jc][:, kk, :],
                                start=(kk == 0),
                                stop=(kk == NK - 1),
                            )
                        if kp == 3:
                            nc.vector.tensor_scalar_add(
                                dst[:, m, QCH * jc : QCH * (jc + 1)],
                                pp[:],
                                b_sb[:, m : m + 1],
                            )
                    return q

                return [make(kp) for kp in range(4)]

            def v_quanta(tb):
                hold = {}

                def make(h):
                    def q():
                        if h == 0:
                            hold["pv"] = psA.tile(
                                [128, CW], f32, tag="pa", name=f"pv{tb}"
                            )
                        pv = hold["pv"]
                        for kk in range(4 * h, 4 * h + 4):
                            nc.tensor.matmul(
                                pv[:],
                                xc[tb // RB][:, kk, 128 * (tb % RB) : 128 * (tb % RB + 1)],
                                wv_sb[:, kk, :],
                                start=(kk == 0),
                                stop=(kk == NK - 1),
                            )
                        if h == 1:
                            nc.vector.tensor_copy(
                                v1_sb[:, tb, :, 0:HD],
                                pv[:].rearrange("p (h d) -> p h d", h=HLOC),
                            )
                    return q

                return [make(h) for h in range(2)]

            def o_quanta(tb, act_evac=False):
                t_sl = slice(128 * tb, 128 * (tb + 1))
                hold = {}

                def make(n):
                    def q():
                        if n == 0:
                            hold["o"] = small.tile(
                                [128, D], f16, tag="osb", bufs=2, name=f"osb{tb}"
                            )
                        o_t = hold["o"]
                        po = psA.tile(
                            [128, 512], f32, tag="pa", name=f"po{tb}{n}"
                        )
                        for m in range(2):
                            nc.tensor.matmul(
                                po[:],
                                attn_q[tb // RB][:, m, 128 * (tb % RB) : 128 * (tb % RB + 1)],
                                wo_sb[:, m, 512 * n : 512 * (n + 1)],
                                start=(m == 0),
                                stop=(m == 1),
                            )
                        if act_evac and n == 1:
                            nc.scalar.copy(o_t[:, 512 * n : 512 * (n + 1)], po[:])
                        else:
                            nc.vector.tensor_copy(
                                o_t[:, 512 * n : 512 * (n + 1)], po[:]
                            )
                        eng = nc.sync if tb % 2 == 0 else nc.gpsimd
                        if act_evac:
                            # tail chunks: ship each half as soon as it is
                            # evacuated so the last DMA drains sooner
                            eng.dma_start(
                                out_d[t_sl, 512 * n : 512 * (n + 1)],
                                o_t[:, 512 * n : 512 * (n + 1)],
                            )
                        elif n == 1:
                            eng.dma_start(out_d[t_sl, :], o_t[:])
                    return q

                return [make(n) for n in range(2)]

            def unit_quanta(c):
                qs = []
                qs += qk_quanta(0, 0, c)
                qs += qk_quanta(0, 1, c)
                for tb in range(RB * c, RB * (c + 1)):
                    qs += v_quanta(tb)
                qs += qk_quanta(1, 0, c)
                qs += qk_quanta(1, 1, c)
                return qs

            # --- attention for (m, j), threading filler quanta between blocks
            def zc_of(i, j):
                ri = i - RB * j
                return 128 * ri if ri > 0 else 0

            def emit_s_block(m, j, i, pt):
                ri = i - RB * j
                zc = zc_of(i, j)
                sp = psS.tile([128, 2, QCH], f32, tag="s",
                              name=f"s{m}{j}{i}")
                for p in range(2):
                    hsl = slice(64 * p, 64 * (p + 1))
                    nc.tensor.matmul(
                        sp[:, p, zc:QCH],
                        kT_sb[hsl, m, 128 * i : 128 * (i + 1)],
                        qT_sb[hsl, m, QCH * j + zc : QCH * (j + 1)],
                        start=True,
                        stop=True,
                    )
                nc.scalar.activation(
                    pt[:, :, zc:QCH], sp[:, :, zc:QCH], AF.Exp, scale=SCALE
                )
                if ri >= 0:
                    # zero the in-window upper triangle: keep where q >= k
                    nc.gpsimd.affine_select(
                        out=pt[:, :, zc : zc + 128],
                        in_=pt[:, :, zc : zc + 128],
                        compare_op=mybir.AluOpType.is_ge,
                        fill=0.0,
                        base=0,
                        pattern=[[0, 2], [1, 128]],
                        channel_multiplier=-1,
                    )

            def emit_attn(m, j, filler, pre=None):
                kb = (j + 1) * RB
                pvp = [
                    psPV.tile([128, QCH], f32, tag=f"pvac{p}",
                              name=f"pvac{m}{j}{p}")
                    for p in range(2)
                ]

                def pv_ap(p, rsl, csl):
                    return pvp[p][rsl, csl]
                pts = {}

                def emit_pv(i, ps=(0, 1)):
                    zc = zc_of(i, j)
                    for p in ps:
                        nc.tensor.matmul(
                            pv_ap(p, slice(0, HD + 1), slice(zc, QCH)),
                            v1_sb[:, i, 2 * m + p, :],
                            pts[i][:, p, zc:QCH],
                            start=(i == 0),
                            stop=(i == kb - 1),
                            skip_group_check=True,
                        )

                def normalize(p):
                    recip = small.tile([1, QCH], f32, tag="recip",
                                       name=f"rc{m}{j}{p}")
                    nc.vector.reciprocal(recip[:], pv_ap(p, slice(HD, HD + 1),
                                                         slice(0, QCH)))
                    bcast = small.tile([64, QCH], f32, tag="bcast", bufs=2,
                                       name=f"bc{m}{j}{p}")
                    nc.gpsimd.partition_broadcast(bcast[:], recip[:])
                    pv_body = pv_ap(p, slice(0, HD), slice(0, QCH))
                    if p == 0:
                        nc.vector.tensor_mul(
                            attn_q[j][0:64, m, :], pv_body, bcast[:]
                        )
                    else:
                        tmp = small.tile([64, QCH], bf, tag="tmp",
                                         name=f"tmp{m}{j}")
                        nc.vector.tensor_mul(tmp[:], pv_body, bcast[:])
                        nc.gpsimd.dma_start(attn_q[j][64:128, m, :], tmp[:])

                if pre is not None:
                    # S/exp already done into persistent tiles: pure PV chase.
                    # p=1 chain first so its normalize + partition-shift DMA
                    # hides under p=0's chain.
                    for i in range(kb):
                        pts[i] = pre[i]
                    for p in (1, 0):
                        for i in range(kb):
                            if i % 8 == 0:
                                f = next(filler, None)
                                if f is not None:
                                    f()
                            emit_pv(i, ps=(p,))
                        normalize(p)
                    return
                else:
                    for i in range(kb):
                        pt = ring.tile([128, 2, QCH], bf, tag="pt",
                                       name=f"pt{m}{j}{i}")
                        emit_s_block(m, j, i, pt)
                        pts[i] = pt
                        f = next(filler, None)
                        if f is not None:
                            f()
                        if i >= LA:
                            emit_pv(i - LA)
                    for i in range(max(0, kb - LA), kb):
                        emit_pv(i)

                for p in (1, 0):
                    normalize(p)

            # --- schedule ----------------------------------------------------
            from itertools import chain

            JL = NQ - 1  # last chunk: S/exp prefetched, PV chased at the end
            pt3 = [
                [
                    pers.tile([128, 2, QCH], bf, tag=f"pt3_{m}_{i}",
                              name=f"pt3_{m}_{i}")
                    for i in range(RB * NQ)
                ]
                for m in range(2)
            ]

            def s3_quanta(m):
                def make(i):
                    return lambda: emit_s_block(m, JL, i, pt3[m][i])
                return [make(i) for i in range(RB * (JL + 1))]

            def o_all(c, act_evac=False):
                return [q for tb in range(RB * c, RB * (c + 1))
                        for q in o_quanta(tb, act_evac)]

            def weave(a, b, ratio=2):
                out, ia, ib = [], 0, 0
                while ia < len(a) or ib < len(b):
                    for _ in range(ratio):
                        if ia < len(a):
                            out.append(a[ia])
                            ia += 1
                    if ib < len(b):
                        out.append(b[ib])
                        ib += 1
                return out

            for q in unit_quanta(0):
                q()
            for c in range(NQ - 1):
                fill = [unit_quanta(c + 1)]
                if c + 1 == JL:
                    # weave in the last chunk's S/exp blocks right after its
                    # Q/K projections so the Act engine never goes idle
                    uq = fill[0]
                    fill = [uq[:8], s3_quanta(0), uq[8:],
                            weave(s3_quanta(1), o_all(c - 1) if c >= 1 else [])]
                elif c >= 1:
                    fill.append(o_all(c - 1))
                filler = chain(*fill)
                emit_attn(0, c, filler)
                emit_attn(1, c, filler)
                for f in filler:
                    f()
            filler = chain(o_all(JL - 1, act_evac=True))
            emit_attn(0, JL, filler, pre=pt3[0])
            emit_attn(1, JL, filler, pre=pt3[1])
            for f in filler:
                f()
            for q in o_all(JL, act_evac=True):
                q()

    nc.compile()
    return nc


def shard_inputs(x, Wq, bq, Wk, bk, Wv, Wo, bf16_in=True):
    import ml_dtypes

    bfi = ml_dtypes.bfloat16
    in_maps = []
    for c in range(NCORES):
        b, g = divmod(c, GROUPS)
        cols = slice(g * CW, (g + 1) * CW)
        in_maps.append(
            {
                "xT": np.ascontiguousarray(x[b].T).astype(bfi),
                "wq": np.ascontiguousarray(Wq[:, cols]).astype(bfi),
                "wk": np.ascontiguousarray(Wk[:, cols]).astype(bfi),
                "wv": np.ascontiguousarray(Wv[:, cols]).astype(bfi),
                "wo": np.ascontiguousarray(Wo[cols, :]).astype(bfi),
                "bq2": np.ascontiguousarray(bq[cols].reshape(CW // 128, 128).T),
                "bk2": np.ascontiguousarray(bk[cols].reshape(CW // 128, 128).T),
            }
        )
    return in_maps


def gather_outputs(results, x, Wv_b, Wo, bo, bv):
    B, T, _ = x.shape
    y = np.empty((B, T, D), np.float32)
    corr = (bv @ Wo + bo).astype(np.float32)
    for b in range(B):
        acc = results[GROUPS * b]["out"].astype(np.float32)
        for g in range(1, GROUPS):
            acc += results[GROUPS * b + g]["out"].astype(np.float32)
        y[b] = acc + corr
    return y


def kernel(x, Wq, bq, Wk, bk, Wv, bv, Wo, bo):
    from concourse import bass_utils

    x = np.asarray(x, np.float32)
    T = x.shape[1]
    if T not in _CACHE:
        _CACHE[T] = build_nc(T)
    nc = _CACHE[T]
    in_maps = shard_inputs(
        x,
        np.asarray(Wq, np.float32), np.asarray(bq, np.float32),
        np.asarray(Wk, np.float32), np.asarray(bk, np.float32),
        np.asarray(Wv, np.float32), np.asarray(Wo, np.float32),
    )
    res = bass_utils.run_bass_kernel_spmd(
        nc, in_maps, core_ids=list(range(NCORES))
    )
    y = gather_outputs(res.results, x, None, np.asarray(Wo, np.float32),
                       np.asarray(bo, np.float32), np.asarray(bv, np.float32))
    return y


# revision 39
# speedup vs baseline: 2.8009x; 1.0490x over previous
"""Trainium2 Bass kernel for classical causal MHA (B=2, T=2048, D=1024, H=16).

Sharding: 8 cores = 2 batches x 4 head-groups (4 heads / 256 dims each).
Each core computes QKV projections for its head-group, causal attention,
and a partial output projection; the host sums the 4 partials per batch
and adds the (bv @ Wo + bo) correction (the v-bias commutes through
softmax-weighted averaging, so it is applied after the kernel).

Design (HW-measured ~97us/rep steady state vs 219us baseline):
- All matmul operands are bf16 (1 cycle/row at any free size, FWL halves
  weight loads, DMA bytes halved). PSUM accumulation stays fp32. NOTE:
  fp16 operands measure ~3x SLOWER on real HW despite the cost model
  rating them equal — do not switch. The partial-output DMA is fp16
  (safe: 10 mantissa bits, values O(1)).
- Score matmuls pack the 2 heads of a pair on disjoint 64-row PE groups
  (auto tile_position) — they genuinely overlap on HW.
- Chunk-pipelined schedule: projections for q-chunk c+1 and the output
  projection for chunk c-1 are split into ~0.4-0.9us quanta and threaded
  between attention blocks of chunk c, so the PE never sits behind the
  activation engine's exp chain. The last chunk's S/exp blocks are
  prefetched into persistent tiles during chunk 2; its PV runs as a
  pure-PE chase at the end (p=1 chain first so the partition-shift DMA
  of its normalize hides under p=0's chain).
- Causal trim: S and PV matmuls stream only the [zc:512) live columns of
  diagonal blocks; the in-window upper triangle is zeroed by a single
  gpsimd affine_select per diagonal block (no masks, no memsets, no DVE
  mask-muls).
- Input DMAs are spread over the sync/scalar/gpsimd queues in dependency
  order; warm-up matmuls (first rep only) keep the PE HAM at 8/8 during
  the initial loads.
- The softmax denominator comes from a ones column appended to V in the
  PV matmul; no max-subtraction is needed because scores are O(1).
"""

import os
import sys

for _p in ("/opt/trn_rl_repo", "/root/.axon_site/_ro/trn_rl_repo"):
    if os.path.isdir(_p) and _p not in sys.path:
        sys.path.insert(0, _p)

import numpy as np

D = 1024
NH = 16
HD = 64
NCORES = 8
GROUPS = 4          # head-groups per batch
HLOC = NH // GROUPS  # heads per core
CW = HLOC * HD       # per-core projection width (256)
SCALE = 1.0 / float(np.sqrt(HD))

_CACHE = {}


def build_nc(T, repeat=1, cfg=None):
    cfg = dict(cfg or {})
    PSA = cfg.get("psA", 2)
    PSS = cfg.get("psS", 2)
    PSPV = cfg.get("psPV", 1)
    RING = cfg.get("ring", 4)
    LA = cfg.get("la", 3)
    WARM = cfg.get("warm", 10)
    import concourse.tile as tile
    from concourse import bacc, mybir

    f32 = mybir.dt.float32
    bf = mybir.dt.bfloat16
    f16 = mybir.dt.float16
    AF = mybir.ActivationFunctionType

    QCH = min(512, T)     # q-chunk width
    NQ = T // QCH
    RB = QCH // 128       # k-blocks per q-chunk
    TB = T // 128
    NK = D // 128         # contraction chunks for projections

    nc = bacc.Bacc(None, target_bir_lowering=False, debug=False)
    xT_d = nc.dram_tensor("xT", [D, T], bf, kind="ExternalInput")
    wq_d = nc.dram_tensor("wq", [D, CW], bf, kind="ExternalInput")
    wk_d = nc.dram_tensor("wk", [D, CW], bf, kind="ExternalInput")
    wv_d = nc.dram_tensor("wv", [D, CW], bf, kind="ExternalInput")
    wo_d = nc.dram_tensor("wo", [CW, D], bf, kind="ExternalInput")
    bq_d = nc.dram_tensor("bq2", [128, CW // 128], f32, kind="ExternalInput")
    bk_d = nc.dram_tensor("bk2", [128, CW // 128], f32, kind="ExternalInput")
    out_d = nc.dram_tensor("out", [T, D], f16, kind="ExternalOutput")

    with tile.TileContext(nc) as tc:
        from contextlib import ExitStack

        for _rep in range(repeat):
          with ExitStack() as es:
            pers = es.enter_context(tc.tile_pool(name=f"pers{_rep}", bufs=1))
            psA = es.enter_context(tc.tile_pool(name=f"psA{_rep}", bufs=PSA, space="PSUM"))
            psS = es.enter_context(tc.tile_pool(name=f"psS{_rep}", bufs=PSS, space="PSUM"))
            psPV = es.enter_context(tc.tile_pool(name=f"psPV{_rep}", bufs=PSPV, space="PSUM"))
            ring = es.enter_context(tc.tile_pool(name=f"ring{_rep}", bufs=RING))
            small = es.enter_context(tc.tile_pool(name=f"small{_rep}", bufs=2))

            qT_sb = pers.tile([128, 2, T], bf, tag="qT")
            kT_sb = pers.tile([128, 2, T], bf, tag="kT")
            attn_q = [
                pers.tile([128, 2, QCH], bf, tag=f"attn{jq}", name=f"attnq{jq}")
                for jq in range(NQ)
            ]
            v1_sb = pers.tile([128, TB, HLOC, HD + 1], bf, tag="v1")
            bq_sb = pers.tile([128, CW // 128], f32, tag="bq")
            bk_sb = pers.tile([128, CW // 128], f32, tag="bk")
            wo_sb = pers.tile([128, 2, D], bf, tag="wo")
            warm_sb = pers.tile([128, 256], bf, tag="warm")
            xc = [
                pers.tile([128, NK, QCH], bf, tag=f"xc{c}", name=f"xc{c}")
                for c in range(NQ)
            ]
            wq_sb = pers.tile([128, NK, CW], bf, tag="wq")
            wk_sb = pers.tile([128, NK, CW], bf, tag="wk")
            wv_sb = pers.tile([128, NK, CW], bf, tag="wv")

            # --- input DMAs, spread across queues in dependency order ---
            def _ld_x(c, eng):
                eng.dma_start(
                    xc[c][:],
                    xT_d[:, QCH * c : QCH * (c + 1)].rearrange(
                        "(k p) t -> p k t", p=128
                    ),
                )

            def _ld_w(w_sb, w_d, eng):
                eng.dma_start(
                    w_sb[:], w_d[:].rearrange("(k p) w -> p k w", p=128)
                )

            # first Q-projection group only needs wq's m=0 half and the low
            # kk-half of x chunk 0 — split those DMAs so compute starts ~2us
            # earlier on a cold dispatch
            nc.sync.dma_start(bq_sb[:], bq_d[:])
            nc.scalar.dma_start(
                wq_sb[:, :, 0:128],
                wq_d[:, 0:128].rearrange("(k p) w -> p k w", p=128),
            )
            nc.gpsimd.dma_start(bk_sb[:], bk_d[:])
            nc.sync.dma_start(
                xc[0][:, 0:4, :],
                xT_d[0:512, 0:QCH].rearrange("(k p) t -> p k t", p=128),
            )
            nc.scalar.dma_start(
                wq_sb[:, :, 128:256],
                wq_d[:, 128:256].rearrange("(k p) w -> p k w", p=128),
            )
            nc.sync.dma_start(
                xc[0][:, 4:8, :],
                xT_d[512:1024, 0:QCH].rearrange("(k p) t -> p k t", p=128),
            )
            _ld_w(wk_sb, wk_d, nc.scalar)
            _ld_w(wv_sb, wv_d, nc.gpsimd)
            _ld_x(1, nc.sync)
            _ld_x(2, nc.sync)
            _ld_x(3, nc.gpsimd)
            for m in range(2):
                nc.gpsimd.dma_start(wo_sb[:, m, :], wo_d[128 * m : 128 * (m + 1), :])

            # ones column of v1 (bf16)
            nc.vector.memset(v1_sb[:, :, :, HD : HD + 1], 1.0)

            # --- PE warm-up during the initial loads (keeps HAM at 8/8).
            # Only the first dispatch starts cold; later reps inherit a warm
            # PE, so don't pay for it in steady state.
            if _rep == 0:
                nc.vector.memset(warm_sb[:], 0.0)
                for w in range(WARM):
                    pw = psA.tile([128, 256], f32, tag="pa", name=f"warm{w}")
                    nc.tensor.matmul(
                        pw[:], warm_sb[:, 0:128], warm_sb[:], start=True,
                        stop=True,
                    )

            # --- work quanta -------------------------------------------------
            def qk_quanta(m, which, jc):
                dst, w_sb, b_sb = (
                    (qT_sb, wq_sb, bq_sb) if which == 0 else (kT_sb, wk_sb, bk_sb)
                )
                hold = {}

                def make(kp):
                    def q():
                        if kp == 0:
                            hold["pp"] = psA.tile(
                                [128, QCH], f32, tag="pa",
                                name=f"pp{m}{which}{jc}",
                            )
                        pp = hold["pp"]
                        for kk in (2 * kp, 2 * kp + 1):
                            nc.tensor.matmul(
                                pp[:],
                                w_sb[:, kk, 128 * m : 128 * (m + 1)],
                                xc[jc][:, kk, :],
                                start=(kk == 0),
                                stop=(kk == NK - 1),
                            )
                        if kp == 3:
                            nc.vector.tensor_scalar_add(
                                dst[:, m, QCH * jc : QCH * (jc + 1)],
                                pp[:],
                                b_sb[:, m : m + 1],
                            )
                    return q

                return [make(kp) for kp in range(4)]

            def v_quanta(tb):
                hold = {}

                def make(h):
                    def q():
                        if h == 0:
                            hold["pv"] = psA.tile(
                                [128, CW], f32, tag="pa", name=f"pv{tb}"
                            )
                        pv = hold["pv"]
                        for kk in range(4 * h, 4 * h + 4):
                            nc.tensor.matmul(
                                pv[:],
                                xc[tb // RB][:, kk, 128 * (tb % RB) : 128 * (tb % RB + 1)],
                                wv_sb[:, kk, :],
                                start=(kk == 0),
                                stop=(kk == NK - 1),
                            )
                        if h == 1:
                            nc.vector.tensor_copy(
                                v1_sb[:, tb, :, 0:HD],
                                pv[:].rearrange("p (h d) -> p h d", h=HLOC),
                            )
                    return q

                return [make(h) for h in range(2)]

            def o_quanta(tb, act_evac=False):
                t_sl = slice(128 * tb, 128 * (tb + 1))
                hold = {}

                def make(n):
                    def q():
                        if n == 0:
                            hold["o"] = small.tile(
                                [128, D], f16, tag="osb", bufs=2, name=f"osb{tb}"
                            )
                        o_t = hold["o"]
                        po = psA.tile(
                            [128, 512], f32, tag="pa", name=f"po{tb}{n}"
                        )
                        for m in range(2):
                            nc.tensor.matmul(
                                po[:],
                                attn_q[tb // RB][:, m, 128 * (tb % RB) : 128 * (tb % RB + 1)],
                                wo_sb[:, m, 512 * n : 512 * (n + 1)],
                                start=(m == 0),
                                stop=(m == 1),
                            )
                        if act_evac and n == 1:
                            nc.scalar.copy(o_t[:, 512 * n : 512 * (n + 1)], po[:])
                        else:
                            nc.vector.tensor_copy(
                                o_t[:, 512 * n : 512 * (n + 1)], po[:]
                            )
                        eng = nc.sync if tb % 2 == 0 else nc.gpsimd
                        if act_evac:
                            # tail chunks: ship each half as soon as it is
                            # evacuated so the last DMA drains sooner
                            eng.dma_start(
                                out_d[t_sl, 512 * n : 512 * (n + 1)],
                                o_t[:, 512 * n : 512 * (n + 1)],
                            )
                        elif n == 1:
                            eng.dma_start(out_d[t_sl, :], o_t[:])
                    return q

                return [make(n) for n in range(2)]

            def unit_quanta(c):
                qs = []
                qs += qk_quanta(0, 0, c)
                qs += qk_quanta(0, 1, c)
                for tb in range(RB * c, RB * (c + 1)):
                    qs += v_quanta(tb)
                qs += qk_quanta(1, 0, c)
                qs += qk_quanta(1, 1, c)
                return qs

            # --- attention for (m, j), threading filler quanta between blocks
            def zc_of(i, j):
                ri = i - RB * j
                return 128 * ri if ri > 0 else 0

            def emit_s_block(m, j, i, pt):
                ri = i - RB * j
                zc = zc_of(i, j)
                sp = psS.tile([128, 2, QCH], f32, tag="s",
                              name=f"s{m}{j}{i}")
                for p in range(2):
                    hsl = slice(64 * p, 64 * (p + 1))
                    nc.tensor.matmul(
                        sp[:, p, zc:QCH],
                        kT_sb[hsl, m, 128 * i : 128 * (i + 1)],
                        qT_sb[hsl, m, QCH * j + zc : QCH * (j + 1)],
                        start=True,
                        stop=True,
                    )
                nc.scalar.activation(
                    pt[:, :, zc:QCH], sp[:, :, zc:QCH], AF.Exp, scale=SCALE
                )
                if ri >= 0:
                    # zero the in-window upper triangle: keep where q >= k
                    nc.gpsimd.affine_select(
                        out=pt[:, :, zc : zc + 128],
                        in_=pt[:, :, zc : zc + 128],
                        compare_op=mybir.AluOpType.is_ge,
                        fill=0.0,
                        base=0,
                        pattern=[[0, 2], [1, 128]],
                        channel_multiplier=-1,
                    )

            def emit_attn(m, j, filler, pre=None):
                kb = (j + 1) * RB
                pvp = [
                    psPV.tile([128, QCH], f32, tag=f"pvac{p}",
                              name=f"pvac{m}{j}{p}")
                    for p in range(2)
                ]

                def pv_ap(p, rsl, csl):
                    return pvp[p][rsl, csl]
                pts = {}

                def emit_pv(i, ps=(0, 1)):
                    zc = zc_of(i, j)
                    for p in ps:
                        nc.tensor.matmul(
                            pv_ap(p, slice(0, HD + 1), slice(zc, QCH)),
                            v1_sb[:, i, 2 * m + p, :],
                            pts[i][:, p, zc:QCH],
                            start=(i == 0),
                            stop=(i == kb - 1),
                            skip_group_check=True,
                        )

                def normalize(p):
                    recip = small.tile([1, QCH], f32, tag="recip",
                                       name=f"rc{m}{j}{p}")
                    nc.vector.reciprocal(recip[:], pv_ap(p, slice(HD, HD + 1),
                                                         slice(0, QCH)))
                    bcast = small.tile([64, QCH], f32, tag="bcast", bufs=2,
                                       name=f"bc{m}{j}{p}")
                    nc.gpsimd.partition_broadcast(bcast[:], recip[:])
                    pv_body = pv_ap(p, slice(0, HD), slice(0, QCH))
                    if p == 0:
                        nc.vector.tensor_mul(
                            attn_q[j][0:64, m, :], pv_body, bcast[:]
                        )
                    else:
                        tmp = small.tile([64, QCH], bf, tag="tmp",
                                         name=f"tmp{m}{j}")
                        nc.vector.tensor_mul(tmp[:], pv_body, bcast[:])
                        nc.gpsimd.dma_start(attn_q[j][64:128, m, :], tmp[:])

                if pre is not None:
                    # S/exp already done into persistent tiles: pure PV chase.
                    # p=1 chain first so its normalize + partition-shift DMA
                    # hides under p=0's chain.
                    for i in range(kb):
                        pts[i] = pre[i]
                    for p in (1, 0):
                        for i in range(kb):
                            if i % 8 == 0:
                                f = next(filler, None)
                                if f is not None:
                                    f()
                            emit_pv(i, ps=(p,))
                        normalize(p)
                    return
                else:
                    for i in range(kb):
                        pt = ring.tile([128, 2, QCH], bf, tag="pt",
                                       name=f"pt{m}{j}{i}")
                        emit_s_block(m, j, i, pt)
                        pts[i] = pt
                        f = next(filler, None)
                        if f is not None:
                            f()
                        if i >= LA:
                            emit_pv(i - LA)
                    for i in range(max(0, kb - LA), kb):
                        emit_pv(i)

                for p in (1, 0):
                    normalize(p)

            # --- schedule ----------------------------------------------------
            from itertools import chain

            JL = NQ - 1  # last chunk: S/exp prefetched, PV chased at the end
            pt3 = [
                [
                    pers.tile([128, 2, QCH], bf, tag=f"pt3_{m}_{i}",
                              name=f"pt3_{m}_{i}")
                    for i in range(RB * NQ)
                ]
                for m in range(2)
            ]

            def s3_quanta(m):
                def make(i):
                    return lambda: emit_s_block(m, JL, i, pt3[m][i])
                return [make(i) for i in range(RB * (JL + 1))]

            def o_all(c, act_evac=False):
                return [q for tb in range(RB * c, RB * (c + 1))
                        for q in o_quanta(tb, act_evac)]

            def weave(a, b, ratio=2):
                out, ia, ib = [], 0, 0
                while ia < len(a) or ib < len(b):
                    for _ in range(ratio):
                        if ia < len(a):
                            out.append(a[ia])
                            ia += 1
                    if ib < len(b):
                        out.append(b[ib])
                        ib += 1
                return out

            for q in unit_quanta(0):
                q()
            for c in range(NQ - 1):
                fill = [unit_quanta(c + 1)]
                if c + 1 == JL:
                    # weave in the last chunk's S/exp blocks right after its
                    # Q/K projections so the Act engine never goes idle
                    uq = fill[0]
                    fill = [uq[:8], s3_quanta(0), uq[8:],
                            weave(s3_quanta(1), o_all(c - 1) if c >= 1 else [])]
                elif c >= 1:
                    fill.append(o_all(c - 1))
                filler = chain(*fill)
                emit_attn(0, c, filler)
                emit_attn(1, c, filler)
                for f in filler:
                    f()
            filler = chain(o_all(JL - 1, act_evac=True))
            emit_attn(0, JL, filler, pre=pt3[0])
            emit_attn(1, JL, filler, pre=pt3[1])
            for f in filler:
                f()
            for q in o_all(JL, act_evac=True):
                q()

    nc.compile()
    return nc


def shard_inputs(x, Wq, bq, Wk, bk, Wv, Wo, bf16_in=True):
    import ml_dtypes

    bfi = ml_dtypes.bfloat16
    in_maps = []
    for c in range(NCORES):
        b, g = divmod(c, GROUPS)
        cols = slice(g * CW, (g + 1) * CW)
        in_maps.append(
            {
                "xT": np.ascontiguousarray(x[b].T).astype(bfi),
                "wq": np.ascontiguousarray(Wq[:, cols]).astype(bfi),
                "wk": np.ascontiguousarray(Wk[:, cols]).astype(bfi),
                "wv": np.ascontiguousarray(Wv[:, cols]).astype(bfi),
                "wo": np.ascontiguousarray(Wo[cols, :]).astype(bfi),
                "bq2": np.ascontiguousarray(bq[cols].reshape(CW // 128, 128).T),
                "bk2": np.ascontiguousarray(bk[cols].reshape(CW // 128, 128).T),
            }
        )
    return in_maps


def gather_outputs(results, x, Wv_b, Wo, bo, bv):
    B, T, _ = x.shape
    y = np.empty((B, T, D), np.float32)
    corr = (bv @ Wo + bo).astype(np.float32)
    for b in range(B):
        acc = results[GROUPS * b]["out"].astype(np.float32)
        for g in range(1, GROUPS):
            acc += results[GROUPS * b + g]["out"].astype(np.float32)
        y[b] = acc + corr
    return y


def kernel(x, Wq, bq, Wk, bk, Wv, bv, Wo, bo):
    from concourse import bass_utils

    x = np.asarray(x, np.float32)
    T = x.shape[1]
    if T not in _CACHE:
        _CACHE[T] = build_nc(T)
    nc = _CACHE[T]
    in_maps = shard_inputs(
        x,
        np.asarray(Wq, np.float32), np.asarray(bq, np.float32),
        np.asarray(Wk, np.float32), np.asarray(bk, np.float32),
        np.asarray(Wv, np.float32), np.asarray(Wo, np.float32),
    )
    res = bass_utils.run_bass_kernel_spmd(
        nc, in_maps, core_ids=list(range(NCORES))
    )
    y = gather_outputs(res.results, x, None, np.asarray(Wo, np.float32),
                       np.asarray(bo, np.float32), np.asarray(bv, np.float32))
    return y


# revision 44
# speedup vs baseline: 2.8131x; 1.0044x over previous
"""Trainium2 Bass kernel for classical causal MHA (B=2, T=2048, D=1024, H=16).

Sharding: 8 cores = 2 batches x 4 head-groups (4 heads / 256 dims each).
Each core computes QKV projections for its head-group, causal attention,
and a partial output projection; the host sums the 4 partials per batch
and adds the (bv @ Wo + bo) correction (the v-bias commutes through
softmax-weighted averaging, so it is applied after the kernel).

Design (HW-measured ~97us/rep steady state vs 219us baseline):
- All matmul operands are bf16 (1 cycle/row at any free size, FWL halves
  weight loads, DMA bytes halved). PSUM accumulation stays fp32. NOTE:
  fp16 operands measure ~3x SLOWER on real HW despite the cost model
  rating them equal — do not switch. The partial-output DMA is fp16
  (safe: 10 mantissa bits, values O(1)).
- Score matmuls pack the 2 heads of a pair on disjoint 64-row PE groups
  (auto tile_position) — they genuinely overlap on HW.
- Chunk-pipelined schedule: projections for q-chunk c+1 and the output
  projection for chunk c-1 are split into ~0.4-0.9us quanta and threaded
  between attention blocks of chunk c, so the PE never sits behind the
  activation engine's exp chain. The last chunk's S/exp blocks are
  prefetched into persistent tiles during chunk 2; its PV runs as a
  pure-PE chase at the end (p=1 chain first so the partition-shift DMA
  of its normalize hides under p=0's chain).
- Causal trim: S and PV matmuls stream only the [zc:512) live columns of
  diagonal blocks; the in-window upper triangle is zeroed by a single
  gpsimd affine_select per diagonal block (no masks, no memsets, no DVE
  mask-muls).
- Input DMAs are spread over the sync/scalar/gpsimd queues in dependency
  order; warm-up matmuls (first rep only) keep the PE HAM at 8/8 during
  the initial loads.
- The softmax denominator comes from a ones column appended to V in the
  PV matmul; no max-subtraction is needed because scores are O(1).
"""

import os
import sys

for _p in ("/opt/trn_rl_repo", "/root/.axon_site/_ro/trn_rl_repo"):
    if os.path.isdir(_p) and _p not in sys.path:
        sys.path.insert(0, _p)

import numpy as np

D = 1024
NH = 16
HD = 64
NCORES = 8
GROUPS = 4          # head-groups per batch
HLOC = NH // GROUPS  # heads per core
CW = HLOC * HD       # per-core projection width (256)
SCALE = 1.0 / float(np.sqrt(HD))

_CACHE = {}


def build_nc(T, repeat=1, cfg=None):
    cfg = dict(cfg or {})
    PSA = cfg.get("psA", 2)
    PSS = cfg.get("psS", 2)
    PSPV = cfg.get("psPV", 1)
    RING = cfg.get("ring", 6)
    LA = cfg.get("la", 4)
    WARM = cfg.get("warm", 10)
    import concourse.tile as tile
    from concourse import bacc, mybir

    f32 = mybir.dt.float32
    bf = mybir.dt.bfloat16
    f16 = mybir.dt.float16
    AF = mybir.ActivationFunctionType

    QCH = min(512, T)     # q-chunk width
    NQ = T // QCH
    RB = QCH // 128       # k-blocks per q-chunk
    TB = T // 128
    NK = D // 128         # contraction chunks for projections

    nc = bacc.Bacc(None, target_bir_lowering=False, debug=False)
    xT_d = nc.dram_tensor("xT", [D, T], bf, kind="ExternalInput")
    wq_d = nc.dram_tensor("wq", [D, CW], bf, kind="ExternalInput")
    wk_d = nc.dram_tensor("wk", [D, CW], bf, kind="ExternalInput")
    wv_d = nc.dram_tensor("wv", [D, CW], bf, kind="ExternalInput")
    wo_d = nc.dram_tensor("wo", [CW, D], bf, kind="ExternalInput")
    bq_d = nc.dram_tensor("bq2", [128, CW // 128], f32, kind="ExternalInput")
    bk_d = nc.dram_tensor("bk2", [128, CW // 128], f32, kind="ExternalInput")
    out_d = nc.dram_tensor("out", [T, D], f16, kind="ExternalOutput")

    with tile.TileContext(nc) as tc:
        from contextlib import ExitStack

        for _rep in range(repeat):
          with ExitStack() as es:
            pers = es.enter_context(tc.tile_pool(name=f"pers{_rep}", bufs=1))
            psA = es.enter_context(tc.tile_pool(name=f"psA{_rep}", bufs=PSA, space="PSUM"))
            psS = es.enter_context(tc.tile_pool(name=f"psS{_rep}", bufs=PSS, space="PSUM"))
            psPV = es.enter_context(tc.tile_pool(name=f"psPV{_rep}", bufs=PSPV, space="PSUM"))
            ring = es.enter_context(tc.tile_pool(name=f"ring{_rep}", bufs=RING))
            small = es.enter_context(tc.tile_pool(name=f"small{_rep}", bufs=2))

            qT_sb = pers.tile([128, 2, T], bf, tag="qT")
            kT_sb = pers.tile([128, 2, T], bf, tag="kT")
            attn_q = [
                pers.tile([128, 2, QCH], bf, tag=f"attn{jq}", name=f"attnq{jq}")
                for jq in range(NQ)
            ]
            v1_sb = pers.tile([128, TB, HLOC, HD + 1], bf, tag="v1")
            bq_sb = pers.tile([128, CW // 128], f32, tag="bq")
            bk_sb = pers.tile([128, CW // 128], f32, tag="bk")
            wo_sb = pers.tile([128, 2, D], bf, tag="wo")
            warm_sb = pers.tile([128, 256], bf, tag="warm")
            xc = [
                pers.tile([128, NK, QCH], bf, tag=f"xc{c}", name=f"xc{c}")
                for c in range(NQ)
            ]
            wq_sb = pers.tile([128, NK, CW], bf, tag="wq")
            wk_sb = pers.tile([128, NK, CW], bf, tag="wk")
            wv_sb = pers.tile([128, NK, CW], bf, tag="wv")

            # --- input DMAs, spread across queues in dependency order ---
            def _ld_x(c, eng):
                eng.dma_start(
                    xc[c][:],
                    xT_d[:, QCH * c : QCH * (c + 1)].rearrange(
                        "(k p) t -> p k t", p=128
                    ),
                )

            def _ld_w(w_sb, w_d, eng):
                eng.dma_start(
                    w_sb[:], w_d[:].rearrange("(k p) w -> p k w", p=128)
                )

            # first Q-projection group only needs wq's m=0 half and the low
            # kk-half of x chunk 0 — split those DMAs so compute starts ~2us
            # earlier on a cold dispatch
            nc.sync.dma_start(bq_sb[:], bq_d[:])
            nc.scalar.dma_start(
                wq_sb[:, :, 0:128],
                wq_d[:, 0:128].rearrange("(k p) w -> p k w", p=128),
            )
            nc.gpsimd.dma_start(bk_sb[:], bk_d[:])
            nc.sync.dma_start(
                xc[0][:, 0:4, :],
                xT_d[0:512, 0:QCH].rearrange("(k p) t -> p k t", p=128),
            )
            nc.scalar.dma_start(
                wq_sb[:, :, 128:256],
                wq_d[:, 128:256].rearrange("(k p) w -> p k w", p=128),
            )
            nc.sync.dma_start(
                xc[0][:, 4:8, :],
                xT_d[512:1024, 0:QCH].rearrange("(k p) t -> p k t", p=128),
            )
            _ld_w(wk_sb, wk_d, nc.scalar)
            _ld_w(wv_sb, wv_d, nc.gpsimd)
            _ld_x(1, nc.sync)
            _ld_x(2, nc.sync)
            _ld_x(3, nc.gpsimd)
            for m in range(2):
                nc.gpsimd.dma_start(wo_sb[:, m, :], wo_d[128 * m : 128 * (m + 1), :])

            # ones column of v1 (bf16)
            nc.vector.memset(v1_sb[:, :, :, HD : HD + 1], 1.0)

            # --- PE warm-up during the initial loads (keeps HAM at 8/8).
            # Only the first dispatch starts cold; later reps inherit a warm
            # PE, so don't pay for it in steady state.
            if _rep == 0:
                nc.vector.memset(warm_sb[:], 0.0)
                for w in range(WARM):
                    pw = psA.tile([128, 256], f32, tag="pa", name=f"warm{w}")
                    nc.tensor.matmul(
                        pw[:], warm_sb[:, 0:128], warm_sb[:], start=True,
                        stop=True,
                    )

            # --- work quanta -------------------------------------------------
            def qk_quanta(m, which, jc):
                dst, w_sb, b_sb = (
                    (qT_sb, wq_sb, bq_sb) if which == 0 else (kT_sb, wk_sb, bk_sb)
                )
                hold = {}

                def make(kp):
                    def q():
                        if kp == 0:
                            hold["pp"] = psA.tile(
                                [128, QCH], f32, tag="pa",
                                name=f"pp{m}{which}{jc}",
                            )
                        pp = hold["pp"]
                        for kk in (2 * kp, 2 * kp + 1):
                            nc.tensor.matmul(
                                pp[:],
                                w_sb[:, kk, 128 * m : 128 * (m + 1)],
                                xc[jc][:, kk, :],
                                start=(kk == 0),
                                stop=(kk == NK - 1),
                            )
                        if kp == 3:
                            nc.vector.tensor_scalar_add(
                                dst[:, m, QCH * jc : QCH * (jc + 1)],
                                pp[:],
                                b_sb[:, m : m + 1],
                            )
                    return q

                return [make(kp) for kp in range(4)]

            def v_quanta(tb):
                hold = {}

                def make(h):
                    def q():
                        if h == 0:
                            hold["pv"] = psA.tile(
                                [128, CW], f32, tag="pa", name=f"pv{tb}"
                            )
                        pv = hold["pv"]
                        for kk in range(4 * h, 4 * h + 4):
                            nc.tensor.matmul(
                                pv[:],
                                xc[tb // RB][:, kk, 128 * (tb % RB) : 128 * (tb % RB + 1)],
                                wv_sb[:, kk, :],
                                start=(kk == 0),
                                stop=(kk == NK - 1),
                            )
                        if h == 1:
                            nc.vector.tensor_copy(
                                v1_sb[:, tb, :, 0:HD],
                                pv[:].rearrange("p (h d) -> p h d", h=HLOC),
                            )
                    return q

                return [make(h) for h in range(2)]

            def o_quanta(tb, act_evac=False):
                t_sl = slice(128 * tb, 128 * (tb + 1))
                hold = {}

                def make(n):
                    def q():
                        if n == 0:
                            hold["o"] = small.tile(
                                [128, D], f16, tag="osb", bufs=2, name=f"osb{tb}"
                            )
                        o_t = hold["o"]
                        po = psA.tile(
                            [128, 512], f32, tag="pa", name=f"po{tb}{n}"
                        )
                        for m in range(2):
                            nc.tensor.matmul(
                                po[:],
                                attn_q[tb // RB][:, m, 128 * (tb % RB) : 128 * (tb % RB + 1)],
                                wo_sb[:, m, 512 * n : 512 * (n + 1)],
                                start=(m == 0),
                                stop=(m == 1),
                            )
                        if act_evac and n == 1:
                            nc.scalar.copy(o_t[:, 512 * n : 512 * (n + 1)], po[:])
                        else:
                            nc.vector.tensor_copy(
                                o_t[:, 512 * n : 512 * (n + 1)], po[:]
                            )
                        eng = nc.sync if tb % 2 == 0 else nc.gpsimd
                        if act_evac:
                            # tail chunks: ship each half as soon as it is
                            # evacuated so the last DMA drains sooner
                            eng.dma_start(
                                out_d[t_sl, 512 * n : 512 * (n + 1)],
                                o_t[:, 512 * n : 512 * (n + 1)],
                            )
                        elif n == 1:
                            eng.dma_start(out_d[t_sl, :], o_t[:])
                    return q

                return [make(n) for n in range(2)]

            def unit_quanta(c):
                qs = []
                qs += qk_quanta(0, 0, c)
                qs += qk_quanta(0, 1, c)
                for tb in range(RB * c, RB * (c + 1)):
                    qs += v_quanta(tb)
                qs += qk_quanta(1, 0, c)
                qs += qk_quanta(1, 1, c)
                return qs

            # --- attention for (m, j), threading filler quanta between blocks
            def zc_of(i, j):
                ri = i - RB * j
                return 128 * ri if ri > 0 else 0

            def emit_s_block(m, j, i, pt):
                ri = i - RB * j
                zc = zc_of(i, j)
                sp = psS.tile([128, 2, QCH], f32, tag="s",
                              name=f"s{m}{j}{i}")
                for p in range(2):
                    hsl = slice(64 * p, 64 * (p + 1))
                    nc.tensor.matmul(
                        sp[:, p, zc:QCH],
                        kT_sb[hsl, m, 128 * i : 128 * (i + 1)],
                        qT_sb[hsl, m, QCH * j + zc : QCH * (j + 1)],
                        start=True,
                        stop=True,
                    )
                nc.scalar.activation(
                    pt[:, :, zc:QCH], sp[:, :, zc:QCH], AF.Exp, scale=SCALE
                )
                if ri >= 0:
                    # zero the in-window upper triangle: keep where q >= k
                    nc.gpsimd.affine_select(
                        out=pt[:, :, zc : zc + 128],
                        in_=pt[:, :, zc : zc + 128],
                        compare_op=mybir.AluOpType.is_ge,
                        fill=0.0,
                        base=0,
                        pattern=[[0, 2], [1, 128]],
                        channel_multiplier=-1,
                    )

            def emit_attn(m, j, filler, pre=None):
                kb = (j + 1) * RB
                pvp = [
                    psPV.tile([128, QCH], f32, tag=f"pvac{p}",
                              name=f"pvac{m}{j}{p}")
                    for p in range(2)
                ]

                def pv_ap(p, rsl, csl):
                    return pvp[p][rsl, csl]
                pts = {}

                def emit_pv(i, ps=(0, 1)):
                    zc = zc_of(i, j)
                    for p in ps:
                        nc.tensor.matmul(
                            pv_ap(p, slice(0, HD + 1), slice(zc, QCH)),
                            v1_sb[:, i, 2 * m + p, :],
                            pts[i][:, p, zc:QCH],
                            start=(i == 0),
                            stop=(i == kb - 1),
                            skip_group_check=True,
                        )

                def normalize(p):
                    recip = small.tile([1, QCH], f32, tag="recip",
                                       name=f"rc{m}{j}{p}")
                    nc.vector.reciprocal(recip[:], pv_ap(p, slice(HD, HD + 1),
                                                         slice(0, QCH)))
                    bcast = small.tile([64, QCH], f32, tag="bcast", bufs=2,
                                       name=f"bc{m}{j}{p}")
                    nc.gpsimd.partition_broadcast(bcast[:], recip[:])
                    pv_body = pv_ap(p, slice(0, HD), slice(0, QCH))
                    if p == 0:
                        nc.vector.tensor_mul(
                            attn_q[j][0:64, m, :], pv_body, bcast[:]
                        )
                    else:
                        tmp = small.tile([64, QCH], bf, tag="tmp",
                                         name=f"tmp{m}{j}")
                        nc.vector.tensor_mul(tmp[:], pv_body, bcast[:])
                        nc.gpsimd.dma_start(attn_q[j][64:128, m, :], tmp[:])

                if pre is not None:
                    # S/exp already done into persistent tiles: pure PV chase.
                    # p=1 chain first so its normalize + partition-shift DMA
                    # hides under p=0's chain.
                    for i in range(kb):
                        pts[i] = pre[i]
                    for p in (1, 0):
                        for i in range(kb):
                            if i % 8 == 0:
                                f = next(filler, None)
                                if f is not None:
                                    f()
                            emit_pv(i, ps=(p,))
                        normalize(p)
                    return
                else:
                    for i in range(kb):
                        pt = ring.tile([128, 2, QCH], bf, tag="pt",
                                       name=f"pt{m}{j}{i}")
                        emit_s_block(m, j, i, pt)
                        pts[i] = pt
                        f = next(filler, None)
                        if f is not None:
                            f()
                        if i >= LA:
                            emit_pv(i - LA)
                    for i in range(max(0, kb - LA), kb):
                        emit_pv(i)

                for p in (1, 0):
                    normalize(p)

            # --- schedule ----------------------------------------------------
            from itertools import chain

            JL = NQ - 1  # last chunk: S/exp prefetched, PV chased at the end
            pt3 = [
                [
                    pers.tile([128, 2, QCH], bf, tag=f"pt3_{m}_{i}",
                              name=f"pt3_{m}_{i}")
                    for i in range(RB * NQ)
                ]
                for m in range(2)
            ]

            def s3_quanta(m):
                def make(i):
                    return lambda: emit_s_block(m, JL, i, pt3[m][i])
                return [make(i) for i in range(RB * (JL + 1))]

            def o_all(c, act_evac=False):
                return [q for tb in range(RB * c, RB * (c + 1))
                        for q in o_quanta(tb, act_evac)]

            def weave(a, b, ratio=2):
                out, ia, ib = [], 0, 0
                while ia < len(a) or ib < len(b):
                    for _ in range(ratio):
                        if ia < len(a):
                            out.append(a[ia])
                            ia += 1
                    if ib < len(b):
                        out.append(b[ib])
                        ib += 1
                return out

            for q in unit_quanta(0):
                q()
            for c in range(NQ - 1):
                fill = [unit_quanta(c + 1)]
                if c + 1 == JL:
                    # weave in the last chunk's S/exp blocks right after its
                    # Q/K projections so the Act engine never goes idle
                    uq = fill[0]
                    fill = [uq[:8], s3_quanta(0), uq[8:],
                            weave(s3_quanta(1), o_all(c - 1) if c >= 1 else [])]
                elif c >= 1:
                    fill.append(o_all(c - 1))
                filler = chain(*fill)
                emit_attn(0, c, filler)
                emit_attn(1, c, filler)
                for f in filler:
                    f()
            filler = chain(o_all(JL - 1, act_evac=True))
            emit_attn(0, JL, filler, pre=pt3[0])
            emit_attn(1, JL, filler, pre=pt3[1])
            for f in filler:
                f()
            for q in o_all(JL, act_evac=True):
                q()

    nc.compile()
    return nc


def shard_inputs(x, Wq, bq, Wk, bk, Wv, Wo, bf16_in=True):
    import ml_dtypes

    bfi = ml_dtypes.bfloat16
    in_maps = []
    for c in range(NCORES):
        b, g = divmod(c, GROUPS)
        cols = slice(g * CW, (g + 1) * CW)
        in_maps.append(
            {
                "xT": np.ascontiguousarray(x[b].T).astype(bfi),
                "wq": np.ascontiguousarray(Wq[:, cols]).astype(bfi),
                "wk": np.ascontiguousarray(Wk[:, cols]).astype(bfi),
                "wv": np.ascontiguousarray(Wv[:, cols]).astype(bfi),
                "wo": np.ascontiguousarray(Wo[cols, :]).astype(bfi),
                "bq2": np.ascontiguousarray(bq[cols].reshape(CW // 128, 128).T),
                "bk2": np.ascontiguousarray(bk[cols].reshape(CW // 128, 128).T),
            }
        )
    return in_maps


def gather_outputs(results, x, Wv_b, Wo, bo, bv):
    B, T, _ = x.shape
    y = np.empty((B, T, D), np.float32)
    corr = (bv @ Wo + bo).astype(np.float32)
    for b in range(B):
        acc = results[GROUPS * b]["out"].astype(np.float32)
        for g in range(1, GROUPS):
            acc += results[GROUPS * b + g]["out"].astype(np.float32)
        y[b] = acc + corr
    return y


def kernel(x, Wq, bq, Wk, bk, Wv, bv, Wo, bo):
    from concourse import bass_utils

    x = np.asarray(x, np.float32)
    T = x.shape[1]
    if T not in _CACHE:
        _CACHE[T] = build_nc(T)
    nc = _CACHE[T]
    in_maps = shard_inputs(
        x,
        np.asarray(Wq, np.float32), np.asarray(bq, np.float32),
        np.asarray(Wk, np.float32), np.asarray(bk, np.float32),
        np.asarray(Wv, np.float32), np.asarray(Wo, np.float32),
    )
    res = bass_utils.run_bass_kernel_spmd(
        nc, in_maps, core_ids=list(range(NCORES))
    )
    y = gather_outputs(res.results, x, None, np.asarray(Wo, np.float32),
                       np.asarray(bo, np.float32), np.asarray(bv, np.float32))
    return y
